# revision 1
# baseline (speedup 1.0000x reference)
"""GAT (graph attention) layer on 8 Trainium2 NeuronCores.

Reference computation (N=8192, F_IN=256, F_OUT=64, alpha=0.2):
    Wh     = h @ W                                  [N, 64]
    f_src  = Wh @ a[:64, 0]                         [N]
    f_dst  = Wh @ a[64:, 0]                         [N]
    e      = leaky_relu(f_src[:,None] + f_dst[None,:], 0.2)
    att    = softmax(where(adj > 0, e, -9e15), axis=1)
    out    = elu(att @ Wh)

Sharding: row-shard the N dimension across 8 cores (1024 query rows per
core); every core computes the full Wh/f_dst (replicated pre-phase).

Algebraic transforms used on-device:
 1. exp(lrelu(u)) = exp(0.2*f_src_i) * exp(0.2*f_dst_j) * exp(0.8*relu(u)).
    The exp(0.2*f_src_i) factor is row-constant and cancels in the softmax
    ratio; exp(0.2*f_dst_j) is folded into the matmul rhs
    (rhs_aug[j,:] = b_j * [Wh_j | 1]).  The trailing ones-column makes the
    attention matmul also produce the softmax denominator Z_i.
 2. exp(0.8*relu(u)) = max(exp(0.8*u), 1), so no separate relu pass:
    one ACT Exp (with the 0.8*f_src_i bias fused via the activation bias
    input, and 0.8*f_dst_j coming from a broadcast row resident in SBUF)
    plus one cheap DVE tensor_scalar max.
 3. Per-window pipeline is a one-directional chain ACT -> DVE -> PE
    (exp -> max/mask-mul -> transpose+matmul), so the in-order engines
    pipeline across windows with no head-of-line stalls.
 4. float32r is used for the transpose + attention-matmul operands: DVE
    writes f32->f32r at full (packed) speed, and the PE runs f32r at
    1-1.5 cycles/row instead of 4 for plain f32.
"""

import sys

sys.path.insert(0, "/opt/trn_rl_repo")

import numpy as np

import concourse.bass as bass  # noqa: F401
import concourse.mybir as mybir
import concourse.tile as tile
from concourse import bacc
from concourse.bass_utils import run_bass_kernel_spmd
from concourse.masks import make_identity

N = 8192
F_IN = 256
F_OUT = 64
N_CORES = 8
ROWS = N // N_CORES  # 1024 query rows per core

F32 = mybir.dt.float32
F32R = mybir.dt.float32r
I32 = mybir.dt.int32
Act = mybir.ActivationFunctionType
Alu = mybir.AluOpType

_CACHE = {}


def _build_nc(repeat=1):
    nc = bacc.Bacc(
        "TRN2",
        target_bir_lowering=False,
        debug=False,
        enable_asserts=False,
        num_devices=N_CORES,
    )

    h = nc.dram_tensor("h", [N, F_IN], F32, kind="ExternalInput")
    hs = nc.dram_tensor("hs", [ROWS, F_IN], F32, kind="ExternalInput")
    adj = nc.dram_tensor("adj", [ROWS, N], I32, kind="ExternalInput")
    W = nc.dram_tensor("W", [F_IN, F_OUT], F32, kind="ExternalInput")
    a = nc.dram_tensor("a", [2 * F_OUT, 1], F32, kind="ExternalInput")
    out = nc.dram_tensor("out", [ROWS, F_OUT], F32, kind="ExternalOutput")

    # DRAM bounce buffer: f_dst column -> free-axis row
    fdd = nc.dram_tensor("fdd", [N], F32)

    from contextlib import nullcontext

    with tile.TileContext(nc) as tc:
        rep_ctx = tc.For_i(0, repeat, 1) if repeat > 1 else nullcontext()
        with rep_ctx:
            _kernel_body(nc, tc, h, hs, adj, W, a, out, fdd)

    nc.compile()
    return nc


def _kernel_body(nc, tc, h, hs, adj, W, a, out, fdd):
    import os
    SKEW = int(os.environ.get("GAT_SKEW", "1"))
    WKBUFS = int(os.environ.get("GAT_WKBUFS", "3"))
    PTBUFS = int(os.environ.get("GAT_PTBUFS", "2"))
    MCH = N // 128  # 64 chunks over all rows
    LCH = ROWS // 128  # 8 local chunks
    WIN = 512  # elementwise working window along j
    NWIN = N // WIN  # 16
    GRP = 4096  # adj DMA granularity along j (2 MB per DMA)
    NGRP = N // GRP  # 2
    WPG = GRP // WIN  # 8 windows per DMA group
    QPW = WIN // 128  # 4 transpose blocks per window

    with (
        tc.tile_pool(name="consts", bufs=1) as consts,
        tc.tile_pool(name="ph", bufs=3) as ph,
        tc.tile_pool(name="adjp", bufs=2) as adjp,
        tc.tile_pool(name="wk", bufs=WKBUFS) as wk,
        tc.tile_pool(name="ep", bufs=2) as ep,
        tc.tile_pool(name="psB", bufs=PTBUFS, space="PSUM") as psB,
    ):
        # ---------------- constants / identities ----------------
        idf = consts.tile([128, 128], F32)
        make_identity(nc, idf)
        idr = consts.tile([128, 128], F32R)
        nc.vector.tensor_copy(idr, idf)

        # Waug = [W | w_src | w_dst], stored as [128, 2, 66]
        Waug = consts.tile([128, 2, 66], F32)
        nc.sync.dma_start(
            out=Waug[:, :, 0:F_OUT],
            in_=W[:, :].rearrange("(c p) f -> p c f", p=128),
        )
        a2 = consts.tile([64, 2], F32)
        nc.sync.dma_start(out=a2[:, 0:1], in_=a[0:F_OUT, :])
        nc.sync.dma_start(out=a2[:, 1:2], in_=a[F_OUT : 2 * F_OUT, :])

        # w_src/w_dst = W^T.T @ a2 pieces: lhsT = W^T chunk [64, 128]
        WTs = consts.tile([64, 2, 128], F32)
        for rc in range(2):
            wtps = psB.tile([64, 128], F32, tag="pt")
            nc.tensor.transpose(wtps, Waug[:, rc, 0:F_OUT], idf)
            nc.any.tensor_copy(WTs[:, rc, :], wtps)
        for rc in range(2):
            wps = psB.tile([128, 2], F32, tag="acc")
            nc.tensor.matmul(wps, lhsT=WTs[:, rc, :], rhs=a2, start=True, stop=True)
            nc.any.tensor_copy(Waug[:, rc, F_OUT : F_OUT + 2], wps)

        # ---------------- WhF = h @ Waug for ALL rows ----------------
        # WhF[:, mc, 0:64] = Wh rows; col 64 = f_src; col 65 = f_dst
        WhF = consts.tile([128, MCH, 66], F32)
        for mc in range(MCH):
            hsb = ph.tile([128, F_IN], F32, tag="hsb")
            nc.sync.dma_start(out=hsb, in_=h[mc * 128 : (mc + 1) * 128, :])
            hT = ph.tile([128, 2, 128], F32, tag="hT")
            for kc in range(2):
                hTps = psB.tile([128, 128], F32, tag="pt")
                nc.tensor.transpose(hTps, hsb[:, kc * 128 : (kc + 1) * 128], idf)
                nc.any.tensor_copy(hT[:, kc, :], hTps)
            whps = psB.tile([128, 66], F32, tag="acc")
            for kc in range(2):
                nc.tensor.matmul(
                    whps,
                    lhsT=hT[:, kc, :],
                    rhs=Waug[:, kc, :],
                    start=(kc == 0),
                    stop=(kc == 1),
                )
            nc.any.tensor_copy(WhF[:, mc, :], whps)

        # f_src for OWN rows (from the h shard input)
        fso = consts.tile([128, LCH], F32)
        for lc in range(LCH):
            hsb2 = ph.tile([128, F_IN], F32, tag="hsb")
            nc.sync.dma_start(out=hsb2, in_=hs[lc * 128 : (lc + 1) * 128, :])
            hT2 = ph.tile([128, 2, 128], F32, tag="hT")
            for kc in range(2):
                hTps2 = psB.tile([128, 128], F32, tag="pt")
                nc.tensor.transpose(hTps2, hsb2[:, kc * 128 : (kc + 1) * 128], idf)
                nc.any.tensor_copy(hT2[:, kc, :], hTps2)
            fops = psB.tile([128, 2], F32, tag="acc")
            for kc in range(2):
                nc.tensor.matmul(
                    fops,
                    lhsT=hT2[:, kc, :],
                    rhs=Waug[:, kc, F_OUT : F_OUT + 2],
                    start=(kc == 0),
                    stop=(kc == 1),
                )
            nc.any.tensor_copy(fso[:, lc : lc + 1], fops[:, 0:1])

        # ---------------- attention-side constants ----------------
        # b_j = exp(0.2 * f_dst_j); rhs_aug[j,:] = b_j * [Wh_j | 1]  (f32r)
        bmat = consts.tile([128, MCH], F32)
        nc.scalar.activation(bmat, WhF[:, :, 65], Act.Exp, bias=0.0, scale=0.2)
        rhs_aug = consts.tile([128, MCH, 68], F32R)
        # zero the 3 pad columns (memset can't write f32r; a x0 tensor_scalar can)
        nc.vector.tensor_scalar(
            rhs_aug[:, :, 65:68], WhF[:, :, 0:3], 0.0, None, Alu.mult
        )
        for mc in range(MCH):
            nc.vector.tensor_scalar(
                rhs_aug[:, mc, 0:F_OUT],
                WhF[:, mc, 0:F_OUT],
                bmat[:, mc : mc + 1],
                None,
                Alu.mult,
            )
        nc.vector.tensor_copy(rhs_aug[:, :, F_OUT], bmat)

        # 0.8-scaled f_src column per local chunk (activation bias input)
        fs08o = consts.tile([128, LCH], F32)
        nc.vector.tensor_scalar(fs08o, fso, 0.8, None, Alu.mult)

        # 0.8*f_dst as a broadcast row: column -> PE transpose -> DRAM ->
        # partition-broadcast DMA into [128, N]
        fd08 = consts.tile([128, MCH], F32)
        nc.vector.tensor_scalar(fd08, WhF[:, :, 65], 0.8, None, Alu.mult)
        fdTps = psB.tile([64, 128], F32, tag="pt")
        nc.tensor.transpose(fdTps, fd08, idf)
        fdTs = consts.tile([64, 128], F32)
        nc.any.tensor_copy(fdTs, fdTps)
        nc.gpsimd.dma_start(out=fdd[:].rearrange("(q p) -> q p", p=128), in_=fdTs)
        fd_bcast = consts.tile([128, N], F32)
        fdd_bc = bass.AP(tensor=fdd, offset=0, ap=[[0, 128], [1, N]])
        nc.gpsimd.dma_start(out=fd_bcast, in_=fdd_bc)

        # ---------------- main loop ----------------
        def issue_mms(acc, pTs_prev, w_prev):
            for q in range(QPW):
                jc = w_prev * QPW + q
                nc.tensor.matmul(
                    acc,
                    lhsT=pTs_prev[:, q, :],
                    rhs=rhs_aug[:, jc, :],
                    start=(w_prev == 0 and q == 0),
                    stop=(w_prev == NWIN - 1 and q == QPW - 1),
                )

        for ic in range(LCH):
            acc = psB.tile([128, 68], F32, tag="acc")
            prev = None
            for wg in range(NGRP):
                adjt = adjp.tile([128, GRP], I32, tag="adj")
                nc.sync.dma_start(
                    out=adjt,
                    in_=adj[ic * 128 : (ic + 1) * 128, wg * GRP : (wg + 1) * GRP],
                )
                for wi in range(WPG):
                    w = wg * WPG + wi
                    # X = exp(0.8*(f_src_i + f_dst_j)); masked & clamped below
                    X = wk.tile([128, WIN], F32, tag="X")
                    nc.scalar.activation(
                        X,
                        fd_bcast[:, w * WIN : (w + 1) * WIN],
                        Act.Exp,
                        bias=fs08o[:, ic : ic + 1],
                        scale=1.0,
                    )
                    Xm = wk.tile([128, WIN], F32, tag="Xm")
                    nc.vector.tensor_scalar(Xm, X, 1.0, None, Alu.max)
                    adjf = wk.tile([128, WIN], F32, tag="adjf")
                    nc.vector.tensor_copy(
                        adjf, adjt[:, wi * WIN : (wi + 1) * WIN]
                    )
                    p = wk.tile([128, WIN], F32R, tag="p")
                    nc.vector.tensor_tensor(p, Xm, adjf, Alu.mult)
                    pTps = psB.tile([128, QPW, 128], F32R, tag="pt")
                    for q in range(QPW):
                        nc.tensor.transpose(
                            pTps[:, q, :], p[:, q * 128 : (q + 1) * 128], idr
                        )
                    pTs = wk.tile([128, QPW, 128], F32R, tag="pts")
                    nc.vector.tensor_copy(pTs, pTps)
                    # software pipelining: the attention matmuls for window
                    # w-1 are issued here, after window w's transposes, so
                    # the PE never stalls waiting for the PSUM->SBUF copy.
                    if SKEW:
                        if prev is not None:
                            issue_mms(acc, *prev)
                        prev = (pTs, w)
                    else:
                        issue_mms(acc, pTs, w)
            if SKEW:
                issue_mms(acc, *prev)
            # epilogue: h' = S / Z ; out = elu(h')
            rz = ep.tile([128, 1], F32, tag="rz")
            nc.vector.reciprocal(rz, acc[:, F_OUT : F_OUT + 1])
            sc = ep.tile([128, F_OUT], F32, tag="sc")
            nc.vector.tensor_scalar(sc, acc[:, 0:F_OUT], rz, None, Alu.mult)
            mn = ep.tile([128, F_OUT], F32, tag="mn")
            nc.vector.tensor_scalar(mn, sc, 0.0, None, Alu.min)
            em = ep.tile([128, F_OUT], F32, tag="em")
            nc.scalar.activation(em, mn, Act.Exp, bias=0.0, scale=1.0)
            rp = ep.tile([128, F_OUT], F32, tag="rp")
            nc.vector.tensor_scalar(rp, sc, 0.0, None, Alu.max)
            s1 = ep.tile([128, F_OUT], F32, tag="s1")
            nc.vector.tensor_tensor(s1, em, rp, Alu.add)
            ob = ep.tile([128, F_OUT], F32, tag="ob")
            nc.vector.tensor_scalar(ob, s1, -1.0, None, Alu.add)
            nc.gpsimd.dma_start(out=out[ic * 128 : (ic + 1) * 128, :], in_=ob)


def _get_nc(repeat=1):
    import os
    key = ("nc", repeat, os.environ.get("GAT_ABLATE", ""), os.environ.get("GAT_SKEW",""), os.environ.get("GAT_WKBUFS",""), os.environ.get("GAT_PTBUFS",""))
    if key not in _CACHE:
        _CACHE[key] = _build_nc(repeat)
    return _CACHE[key]


def kernel(h, adj, W, a, _collect_results=False, _trace=False):
    h = np.ascontiguousarray(h, dtype=np.float32)
    adj = np.ascontiguousarray(adj, dtype=np.int32)
    W = np.ascontiguousarray(W, dtype=np.float32)
    a = np.ascontiguousarray(a, dtype=np.float32)

    nc = _get_nc()
    in_maps = []
    for c in range(N_CORES):
        sl = slice(c * ROWS, (c + 1) * ROWS)
        in_maps.append(
            {
                "h": h,
                "hs": np.ascontiguousarray(h[sl]),
                "adj": np.ascontiguousarray(adj[sl]),
                "W": W,
                "a": a,
            }
        )
    res = run_bass_kernel_spmd(nc, in_maps, list(range(N_CORES)), trace=_trace)
    out = np.concatenate([res.results[c]["out"] for c in range(N_CORES)], axis=0)
    out = np.ascontiguousarray(out, dtype=np.float32)
    if _collect_results:
        return out, res
    return out



# revision 3
# speedup vs baseline: 2.5638x; 2.5638x over previous
"""GAT (graph attention) layer on 8 Trainium2 NeuronCores.

Reference computation (N=8192, F_IN=256, F_OUT=64, alpha=0.2):
    Wh     = h @ W                                  [N, 64]
    f_src  = Wh @ a[:64, 0]                         [N]
    f_dst  = Wh @ a[64:, 0]                         [N]
    e      = leaky_relu(f_src[:,None] + f_dst[None,:], 0.2)
    att    = softmax(where(adj > 0, e, -9e15), axis=1)
    out    = elu(att @ Wh)

Sharding: row-shard the N dimension across 8 cores (1024 query rows per
core).  During host-side sharding each core's adj row-block is staged
TRANSPOSED (adjT[j, i] = adj[i, j], contiguous [8192, 1024] int32) and h
is staged as hT = h.T, so the device kernel never needs PE transposes in
its hot loop.

Algebraic transforms (same softmax factorization as proven correct by the
earlier row-major version):
 1. exp(lrelu(u)) = exp(0.2*f_src_i) * exp(0.2*f_dst_j) * exp(0.8*relu(u)).
    The exp(0.2*f_src_i) factor is row-constant and cancels in the softmax
    ratio; exp(0.2*f_dst_j) is folded into the stationary matmul operand
    (rhs_aug[j,:] = b_j * [Wh_j | 1]).  The trailing ones-column makes the
    attention matmul also produce the softmax denominator Z_i.
 2. exp(0.8*relu(u)) = max(exp(0.8*u), 1): one ACT Exp + the max folded
    into a single fused DVE scalar_tensor_tensor with the adj mask:
    p = (X max 1) * adj.
 3. The attention matrix is generated directly in TRANSPOSED form
    pT[j, i]: f_dst_j is the per-partition ACT bias, f_src_i a broadcast
    row.  The attention matmul is then
        accT[f, i] = sum_j rhs_aug[j, f] * pT[j, i]
    (lhsT = rhs_aug stationary, pT the 512-wide f32r moving operand at
    1 cycle/row) with zero transposes and f32 PSUM accumulation.

Per-window pipeline: DMA (adjT) -> ACT (exp) -> DVE (fused max*mask) ->
PE (2 matmuls), all windows independent, so the kernel runs at the adj
streaming rate (memory-bound).
"""

import sys

sys.path.insert(0, "/opt/trn_rl_repo")

import numpy as np

import concourse.bass as bass  # noqa: F401
import concourse.mybir as mybir
import concourse.tile as tile
from concourse import bacc
from concourse.bass_utils import run_bass_kernel_spmd
from concourse.masks import make_identity

N = 8192
F_IN = 256
F_OUT = 64
N_CORES = 8
ROWS = N // N_CORES  # 1024 query rows per core
KC = F_IN // 128  # 2 contraction chunks
MCH = N // 128  # 64 j-chunks
LCH = ROWS // 128  # 8 local row chunks

F32 = mybir.dt.float32
F32R = mybir.dt.float32r
I32 = mybir.dt.int32
Act = mybir.ActivationFunctionType
Alu = mybir.AluOpType

_CACHE = {}


def _build_nc(repeat=1):
    nc = bacc.Bacc(
        "TRN2",
        target_bir_lowering=False,
        debug=False,
        enable_asserts=False,
        num_devices=N_CORES,
    )

    hT = nc.dram_tensor("hT", [F_IN, N], F32, kind="ExternalInput")
    hTs = nc.dram_tensor("hTs", [F_IN, ROWS], F32, kind="ExternalInput")
    adjT = nc.dram_tensor("adjT", [N, ROWS], I32, kind="ExternalInput")
    W = nc.dram_tensor("W", [F_IN, F_OUT], F32, kind="ExternalInput")
    a = nc.dram_tensor("a", [2 * F_OUT, 1], F32, kind="ExternalInput")
    out = nc.dram_tensor("out", [ROWS, F_OUT], F32, kind="ExternalOutput")

    # DRAM bounce buffer: f_src column -> free-axis row broadcast
    fsd = nc.dram_tensor("fsd", [ROWS], F32)

    from contextlib import nullcontext

    with tile.TileContext(nc) as tc:
        rep_ctx = tc.For_i(0, repeat, 1) if repeat > 1 else nullcontext()
        with rep_ctx:
            _kernel_body(nc, tc, hT, hTs, adjT, W, a, out, fsd)

    nc.compile()
    return nc


def _kernel_body(nc, tc, hT, hTs, adjT, W, a, out, fsd):
    import os

    ADJBUFS = int(os.environ.get("GAT_ADJBUFS", "4"))
    WKBUFS = int(os.environ.get("GAT_WKBUFS", "3"))

    with (
        tc.tile_pool(name="consts", bufs=1) as consts,
        tc.tile_pool(name="pre", bufs=2) as pre,
        tc.tile_pool(name="adjp", bufs=ADJBUFS) as adjp,
        tc.tile_pool(name="wk", bufs=WKBUFS) as wk,
        tc.tile_pool(name="ep", bufs=2) as ep,
        tc.tile_pool(name="psA", bufs=2, space="PSUM") as psA,
        tc.tile_pool(name="psAcc", bufs=1, space="PSUM") as psAcc,
    ):
        # ---------------- constants ----------------
        idf = consts.tile([128, 128], F32)
        make_identity(nc, idf)

        # hT (all rows) and hTs (own rows), both [128, KC, n] layouts.
        hTa = consts.tile([128, KC, N], F32)
        for g in range(8):
            sl = slice(g * 1024, (g + 1) * 1024)
            nc.gpsimd.dma_start(
                out=hTa[:, :, sl],
                in_=hT[:, sl].rearrange("(c p) n -> p c n", p=128),
            )
        hTo = consts.tile([128, KC, ROWS], F32)
        nc.gpsimd.dma_start(
            out=hTo, in_=hTs[:, :].rearrange("(c p) n -> p c n", p=128)
        )

        # a2s = [0.8*a_src | a_dst] as [64, 2]
        a2 = consts.tile([64, 2], F32)
        nc.sync.dma_start(out=a2[:, 0:1], in_=a[0:F_OUT, :])
        nc.sync.dma_start(out=a2[:, 1:2], in_=a[F_OUT : 2 * F_OUT, :])
        a2s = consts.tile([64, 2], F32)
        nc.vector.tensor_scalar(a2s[:, 0:1], a2[:, 0:1], 0.8, None, Alu.mult)
        nc.vector.tensor_copy(a2s[:, 1:2], a2[:, 1:2])

        # Waug = [W | w_dst] ([128, KC, 65]); wsrc08 = 0.8*w_src ([128, KC])
        Waug = consts.tile([128, KC, F_OUT + 1], F32)
        nc.sync.dma_start(
            out=Waug[:, :, 0:F_OUT],
            in_=W[:, :].rearrange("(c p) f -> p c f", p=128),
        )
        WTs = consts.tile([64, KC, 128], F32)
        for rc in range(KC):
            wtps = psA.tile([64, 128], F32, tag="pt")
            nc.tensor.transpose(wtps, Waug[:, rc, 0:F_OUT], idf)
            nc.any.tensor_copy(WTs[:, rc, :], wtps)
        wsrc08 = consts.tile([128, KC], F32)
        for rc in range(KC):
            wps = psA.tile([128, 2], F32, tag="pt")
            nc.tensor.matmul(wps, lhsT=WTs[:, rc, :], rhs=a2s, start=True, stop=True)
            nc.any.tensor_copy(wsrc08[:, rc : rc + 1], wps[:, 0:1])
            nc.any.tensor_copy(Waug[:, rc, F_OUT : F_OUT + 1], wps[:, 1:2])

        # ---------------- WhF = [Wh | f_dst] for ALL rows ----------------
        WhF = consts.tile([128, MCH, F_OUT + 1], F32)
        for mc in range(MCH):
            whps = psA.tile([128, F_OUT + 1], F32, tag="acc")
            for kc in range(KC):
                nc.tensor.matmul(
                    whps,
                    lhsT=hTa[:, kc, mc * 128 : (mc + 1) * 128],
                    rhs=Waug[:, kc, :],
                    start=(kc == 0),
                    stop=(kc == KC - 1),
                )
            nc.any.tensor_copy(WhF[:, mc, :], whps)

        # f_src08 for OWN rows -> column [128, LCH]
        fso = consts.tile([128, LCH], F32)
        for ic in range(LCH):
            fps = psA.tile([128, 1], F32, tag="pt")
            for kc in range(KC):
                nc.tensor.matmul(
                    fps,
                    lhsT=hTo[:, kc, ic * 128 : (ic + 1) * 128],
                    rhs=wsrc08[:, kc : kc + 1],
                    start=(kc == 0),
                    stop=(kc == KC - 1),
                )
            nc.any.tensor_copy(fso[:, ic : ic + 1], fps)

        # f_src08 column -> broadcast row [128, ROWS] (PE transpose + DRAM
        # bounce + partition-broadcast DMA)
        fsTps = psA.tile([LCH, 128], F32, tag="pt")
        nc.tensor.transpose(fsTps, fso, idf)
        fsTs = pre.tile([LCH, 128], F32, tag="fsT")
        nc.any.tensor_copy(fsTs, fsTps)
        nc.gpsimd.dma_start(out=fsd[:].rearrange("(q p) -> q p", p=128), in_=fsTs)
        fs08row = consts.tile([128, ROWS], F32)
        fsd_bc = bass.AP(tensor=fsd, offset=0, ap=[[0, 128], [1, ROWS]])
        nc.gpsimd.dma_start(out=fs08row, in_=fsd_bc)

        # ---------------- attention-side constants ----------------
        # b_j = exp(0.2*f_dst_j); fd08_j = 0.8*f_dst_j
        bmat = consts.tile([128, MCH], F32)
        nc.scalar.activation(bmat, WhF[:, :, F_OUT], Act.Exp, bias=0.0, scale=0.2)
        fd08 = consts.tile([128, MCH], F32)
        nc.vector.tensor_scalar(fd08, WhF[:, :, F_OUT], 0.8, None, Alu.mult)
        # rhs_aug[j, :] = b_j * [Wh_j | 1]  (f32r stationary operand)
        rhs_aug = consts.tile([128, MCH, F_OUT + 1], F32R)
        for mc in range(MCH):
            nc.vector.tensor_scalar(
                rhs_aug[:, mc, 0:F_OUT],
                WhF[:, mc, 0:F_OUT],
                bmat[:, mc : mc + 1],
                None,
                Alu.mult,
            )
        nc.vector.tensor_copy(rhs_aug[:, :, F_OUT], bmat)

        # ---------------- main loop over j-chunks ----------------
        acc0 = psAcc.tile([F_OUT + 1, 512], F32, tag="a0")
        acc1 = psAcc.tile([F_OUT + 1, 512], F32, tag="a1")
        for mc in range(MCH):
            adjt = adjp.tile([128, ROWS], I32, tag="adj")
            nc.sync.dma_start(out=adjt, in_=adjT[mc * 128 : (mc + 1) * 128, :])
            # X = exp(0.8*f_src_i + 0.8*f_dst_j)   [128 j, 1024 i]
            X = wk.tile([128, ROWS], F32, tag="X")
            nc.scalar.activation(
                X, fs08row, Act.Exp, bias=fd08[:, mc : mc + 1], scale=1.0
            )
            # pT = max(X, 1) * adj   (fused, f32r)
            p = wk.tile([128, ROWS], F32R, tag="p")
            nc.vector.scalar_tensor_tensor(p, X, 1.0, adjt, Alu.max, Alu.mult)
            nc.tensor.matmul(
                acc0,
                lhsT=rhs_aug[:, mc, :],
                rhs=p[:, 0:512],
                start=(mc == 0),
                stop=(mc == MCH - 1),
            )
            nc.tensor.matmul(
                acc1,
                lhsT=rhs_aug[:, mc, :],
                rhs=p[:, 512:1024],
                start=(mc == 0),
                stop=(mc == MCH - 1),
            )

        # ---------------- epilogue: transpose accT, divide, elu ----------------
        accS = ep.tile([F_OUT + 1, ROWS], F32, tag="accS")
        nc.any.tensor_copy(accS[:, 0:512], acc0)
        nc.any.tensor_copy(accS[:, 512:1024], acc1)
        for q in range(LCH):
            trp = psA.tile([128, F_OUT + 1], F32, tag="tr")
            nc.tensor.transpose(
                trp,
                accS[:, q * 128 : (q + 1) * 128],
                idf[0 : F_OUT + 1, 0 : F_OUT + 1],
            )
            sc = ep.tile([128, F_OUT + 1], F32, tag="sc")
            nc.any.tensor_copy(sc, trp)
            rz = ep.tile([128, 1], F32, tag="rz")
            nc.vector.reciprocal(rz, sc[:, F_OUT : F_OUT + 1])
            hp = ep.tile([128, F_OUT], F32, tag="hp")
            nc.vector.tensor_scalar(hp, sc[:, 0:F_OUT], rz, None, Alu.mult)
            # elu(x) = max(x,0) + exp(min(x,0)) - 1
            mn = ep.tile([128, F_OUT], F32, tag="mn")
            nc.vector.tensor_scalar(mn, hp, 0.0, None, Alu.min)
            em = ep.tile([128, F_OUT], F32, tag="em")
            nc.scalar.activation(em, mn, Act.Exp, bias=0.0, scale=1.0)
            rp = ep.tile([128, F_OUT], F32, tag="rp")
            nc.vector.tensor_scalar(rp, hp, 0.0, None, Alu.max)
            s1 = ep.tile([128, F_OUT], F32, tag="s1")
            nc.vector.tensor_tensor(s1, em, rp, Alu.add)
            ob = ep.tile([128, F_OUT], F32, tag="ob")
            nc.vector.tensor_scalar(ob, s1, -1.0, None, Alu.add)
            nc.gpsimd.dma_start(out=out[q * 128 : (q + 1) * 128, :], in_=ob)


def _get_nc(repeat=1):
    import os

    key = (
        "nc",
        repeat,
        os.environ.get("GAT_ADJBUFS", ""),
        os.environ.get("GAT_WKBUFS", ""),
    )
    if key not in _CACHE:
        _CACHE[key] = _build_nc(repeat)
    return _CACHE[key]


def _make_in_maps(h, adj, W, a):
    h = np.ascontiguousarray(h, dtype=np.float32)
    adj = np.ascontiguousarray(adj, dtype=np.int32)
    W = np.ascontiguousarray(W, dtype=np.float32)
    a = np.ascontiguousarray(a, dtype=np.float32)
    hT = np.ascontiguousarray(h.T)
    in_maps = []
    for c in range(N_CORES):
        sl = slice(c * ROWS, (c + 1) * ROWS)
        in_maps.append(
            {
                "hT": hT,
                "hTs": np.ascontiguousarray(hT[:, sl]),
                "adjT": np.ascontiguousarray(adj[sl].T),
                "W": W,
                "a": a,
            }
        )
    return in_maps


def kernel(h, adj, W, a, _collect_results=False, _trace=False):
    in_maps = _make_in_maps(h, adj, W, a)
    nc = _get_nc()
    res = run_bass_kernel_spmd(nc, in_maps, list(range(N_CORES)), trace=_trace)
    out = np.concatenate([res.results[c]["out"] for c in range(N_CORES)], axis=0)
    out = np.ascontiguousarray(out, dtype=np.float32)
    if _collect_results:
        return out, res
    return out


# revision 6
# speedup vs baseline: 2.5969x; 1.0129x over previous
"""GAT (graph attention) layer on 8 Trainium2 NeuronCores.

Reference computation (N=8192, F_IN=256, F_OUT=64, alpha=0.2):
    Wh     = h @ W                                  [N, 64]
    f_src  = Wh @ a[:64, 0]                         [N]
    f_dst  = Wh @ a[64:, 0]                         [N]
    e      = leaky_relu(f_src[:,None] + f_dst[None,:], 0.2)
    att    = softmax(where(adj > 0, e, -9e15), axis=1)
    out    = elu(att @ Wh)

Sharding: row-shard the N dimension across 8 cores (1024 query rows per
core).  During host-side sharding each core's adj row-block is staged
TRANSPOSED (adjT[j, i] = adj[i, j], contiguous [8192, 1024] int32) and h
is staged as hT = h.T, so the device kernel needs no PE transposes in
its hot loop.

Algebraic transforms (softmax factorization identical to the proven
row-major version):
 1. exp(lrelu(u)) = exp(0.2*f_src_i) * exp(0.2*f_dst_j) * exp(0.8*relu(u)).
    exp(0.2*f_src_i) is row-constant and cancels in the softmax ratio;
    b_j = exp(0.2*f_dst_j) is folded into the stationary matmul operand
    rhs_aug[j,:] = b_j * [Wh_j | 1].  The ones-column makes the attention
    matmul also produce the softmax denominator Z_i.
 2. exp(0.8*relu(u)) = max(exp(0.8*u), 1): one ACT Exp plus the max and
    the adjacency mask fused into a single DVE/Pool scalar_tensor_tensor:
    p = (X max 1) * adj.
 3. The attention matrix is generated directly TRANSPOSED, pT[j, i]:
    f_dst_j is the per-partition ACT bias, f_src_i a broadcast row.  The
    attention matmul is accT[f, i] = sum_j rhs_aug[j, f] * pT[j, i]
    (stationary lhsT = rhs_aug, 512-wide f32r moving pT at 1 cycle/row),
    f32 PSUM accumulation across all 64 j-chunks.

The Wh pre-phase is interleaved with the main loop group-by-group and
rhs_aug is built directly out of PSUM, so the adj DMA stream (the
memory-bound resource) never stalls.
"""

import sys

sys.path.insert(0, "/opt/trn_rl_repo")

import numpy as np

import concourse.bass as bass  # noqa: F401
import concourse.mybir as mybir
import concourse.tile as tile
from concourse import bacc
from concourse.bass_utils import run_bass_kernel_spmd
from concourse.masks import make_identity

N = 8192
F_IN = 256
F_OUT = 64
N_CORES = 8
ROWS = N // N_CORES  # 1024 query rows per core
KC = F_IN // 128  # 2 contraction chunks
MCH = N // 128  # 64 j-chunks
LCH = ROWS // 128  # 8 local row chunks
GRP = MCH // 8  # j-chunks per pre-phase group

F32 = mybir.dt.float32
F32R = mybir.dt.float32r
I32 = mybir.dt.int32
Act = mybir.ActivationFunctionType
Alu = mybir.AluOpType

_CACHE = {}


def _build_nc(repeat=1):
    nc = bacc.Bacc(
        "TRN2",
        target_bir_lowering=False,
        debug=False,
        enable_asserts=False,
        num_devices=N_CORES,
    )

    hT = nc.dram_tensor("hT", [F_IN, N], F32, kind="ExternalInput")
    hTs = nc.dram_tensor("hTs", [F_IN, ROWS], F32, kind="ExternalInput")
    adjT = nc.dram_tensor("adjT", [N, ROWS], I32, kind="ExternalInput")
    W = nc.dram_tensor("W", [F_IN, F_OUT], F32, kind="ExternalInput")
    a = nc.dram_tensor("a", [2 * F_OUT, 1], F32, kind="ExternalInput")
    out = nc.dram_tensor("out", [ROWS, F_OUT], F32, kind="ExternalOutput")

    # DRAM bounce buffer: f_src column -> free-axis row broadcast
    fsd = nc.dram_tensor("fsd", [ROWS], F32)

    from contextlib import nullcontext

    with tile.TileContext(nc) as tc:
        rep_ctx = tc.For_i(0, repeat, 1) if repeat > 1 else nullcontext()
        with rep_ctx:
            _kernel_body(nc, tc, hT, hTs, adjT, W, a, out, fsd)

    nc.compile()
    return nc


def _kernel_body(nc, tc, hT, hTs, adjT, W, a, out, fsd):
    import os

    ADJBUFS = int(os.environ.get("GAT_ADJBUFS", "14"))
    XBUFS = int(os.environ.get("GAT_XBUFS", "3"))
    PBUFS = int(os.environ.get("GAT_PBUFS", "4"))
    # NOTE: gpsimd does not support TensorScalarPtr on real HW (codegen
    # rejects it), so the fused-op offload stays disabled by default.
    POOLWIN = int(os.environ.get("GAT_POOLWIN", "0"))  # windows on gpsimd
    # windows handled by gpsimd, spread evenly
    if POOLWIN > 0:
        stride = MCH / POOLWIN
        pool_set = {int(stride * k + stride / 2) for k in range(POOLWIN)}
    else:
        pool_set = set()

    with (
        tc.tile_pool(name="consts", bufs=1) as consts,
        tc.tile_pool(name="hp", bufs=3) as hp,
        tc.tile_pool(name="adjp", bufs=ADJBUFS) as adjp,
        tc.tile_pool(name="xk", bufs=XBUFS) as xk,
        tc.tile_pool(name="pk", bufs=PBUFS) as pk,
        tc.tile_pool(name="ep", bufs=2) as ep,
        tc.tile_pool(name="psW", bufs=2, space="PSUM") as psW,
        tc.tile_pool(name="psF", bufs=2, space="PSUM") as psF,
        tc.tile_pool(name="psM", bufs=2, space="PSUM") as psM,
        tc.tile_pool(name="psAcc", bufs=1, space="PSUM") as psAcc,
    ):
        # ---------------- small constants ----------------
        idf = consts.tile([128, 128], F32)
        make_identity(nc, idf)

        # a2s = [0.8*a_src | a_dst] as [64, 2]
        a2 = consts.tile([64, 2], F32)
        nc.gpsimd.dma_start(out=a2[:, 0:1], in_=a[0:F_OUT, :])
        nc.gpsimd.dma_start(out=a2[:, 1:2], in_=a[F_OUT : 2 * F_OUT, :])
        a2s = consts.tile([64, 2], F32)
        nc.vector.tensor_scalar(a2s[:, 0:1], a2[:, 0:1], 0.8, None, Alu.mult)
        nc.vector.tensor_copy(a2s[:, 1:2], a2[:, 1:2])

        # Waug = [W | w_dst] ([128, KC, 65]); wsrc08 = 0.8*w_src ([128, KC])
        Waug = consts.tile([128, KC, F_OUT + 1], F32)
        nc.gpsimd.dma_start(
            out=Waug[:, :, 0:F_OUT],
            in_=W[:, :].rearrange("(c p) f -> p c f", p=128),
        )
        WTs = consts.tile([64, KC, 128], F32)
        for rc in range(KC):
            wtps = psM.tile([64, 128], F32, tag="m")
            nc.tensor.transpose(wtps, Waug[:, rc, 0:F_OUT], idf)
            nc.any.tensor_copy(WTs[:, rc, :], wtps)
        wsrc08 = consts.tile([128, KC], F32)
        for rc in range(KC):
            wps = psM.tile([128, 2], F32, tag="m")
            nc.tensor.matmul(wps, lhsT=WTs[:, rc, :], rhs=a2s, start=True, stop=True)
            nc.any.tensor_copy(wsrc08[:, rc : rc + 1], wps[:, 0:1])
            nc.any.tensor_copy(Waug[:, rc, F_OUT : F_OUT + 1], wps[:, 1:2])

        # ---------------- own-rows f_src08 -> broadcast row ----------------
        hTo = consts.tile([128, KC, ROWS], F32)
        nc.gpsimd.dma_start(
            out=hTo, in_=hTs[:, :].rearrange("(c p) n -> p c n", p=128)
        )
        fps = psM.tile([128, LCH], F32, tag="m")
        for ic in range(LCH):
            for kc in range(KC):
                nc.tensor.matmul(
                    fps[:, ic : ic + 1],
                    lhsT=hTo[:, kc, ic * 128 : (ic + 1) * 128],
                    rhs=wsrc08[:, kc : kc + 1],
                    start=(kc == 0),
                    stop=(kc == KC - 1),
                )
        fso = consts.tile([128, LCH], F32)
        nc.any.tensor_copy(fso, fps)
        fsTps = psM.tile([LCH, 128], F32, tag="m")
        nc.tensor.transpose(fsTps, fso, idf)
        fsTs = consts.tile([LCH, 128], F32)
        nc.any.tensor_copy(fsTs, fsTps)
        nc.gpsimd.dma_start(out=fsd[:].rearrange("(q p) -> q p", p=128), in_=fsTs)
        fs08row = consts.tile([128, ROWS], F32)
        fsd_bc = bass.AP(tensor=fsd, offset=0, ap=[[0, 128], [1, ROWS]])
        nc.gpsimd.dma_start(out=fs08row, in_=fsd_bc)

        # ---------------- persistent attention-side tiles ----------------
        bmat = consts.tile([128, MCH], F32)
        fd08 = consts.tile([128, MCH], F32)
        rhs_aug = consts.tile([128, MCH, F_OUT + 1], F32R)

        acc0 = psAcc.tile([F_OUT + 1, 512], F32, tag="a0")
        acc1 = psAcc.tile([F_OUT + 1, 512], F32, tag="a1")

        # ---------------- main loop: pre-phase interleaved by group ----------------
        for g in range(8):
            gsl = slice(g * 1024, (g + 1) * 1024)
            # hT chunk for this group's 8 j-chunks
            hTg = hp.tile([128, KC, 1024], F32, tag="hTg")
            nc.gpsimd.dma_start(
                out=hTg, in_=hT[:, gsl].rearrange("(c p) n -> p c n", p=128)
            )
            # f_dst for the group's 8 chunks, batched into one PSUM tile
            fdps = psF.tile([128, GRP], F32, tag="fd")
            for q in range(GRP):
                for kc in range(KC):
                    nc.tensor.matmul(
                        fdps[:, q : q + 1],
                        lhsT=hTg[:, kc, q * 128 : (q + 1) * 128],
                        rhs=Waug[:, kc, F_OUT : F_OUT + 1],
                        start=(kc == 0),
                        stop=(kc == KC - 1),
                    )
            gs = slice(g * GRP, (g + 1) * GRP)
            nc.scalar.activation(bmat[:, gs], fdps, Act.Exp, bias=0.0, scale=0.2)
            nc.vector.tensor_scalar(fd08[:, gs], fdps, 0.8, None, Alu.mult)
            nc.vector.tensor_copy(rhs_aug[:, gs, F_OUT], bmat[:, gs])

            for q in range(GRP):
                mc = g * GRP + q
                # Wh chunk -> PSUM; rhs_aug built straight from PSUM
                whps = psW.tile([128, F_OUT], F32, tag="wh")
                for kc in range(KC):
                    nc.tensor.matmul(
                        whps,
                        lhsT=hTg[:, kc, q * 128 : (q + 1) * 128],
                        rhs=Waug[:, kc, 0:F_OUT],
                        start=(kc == 0),
                        stop=(kc == KC - 1),
                    )
                nc.vector.tensor_scalar(
                    rhs_aug[:, mc, 0:F_OUT],
                    whps,
                    bmat[:, mc : mc + 1],
                    None,
                    Alu.mult,
                )

                # ---- attention window mc ----
                adjt = adjp.tile([128, ROWS], I32, tag="adj")
                nc.sync.dma_start(
                    out=adjt, in_=adjT[mc * 128 : (mc + 1) * 128, :]
                )
                X = xk.tile([128, ROWS], F32, tag="X")
                nc.scalar.activation(
                    X, fs08row, Act.Exp, bias=fd08[:, mc : mc + 1], scale=1.0
                )
                p = pk.tile([128, ROWS], F32R, tag="p")
                eng = nc.gpsimd if mc in pool_set else nc.vector
                eng.scalar_tensor_tensor(p, X, 1.0, adjt, Alu.max, Alu.mult)
                nc.tensor.matmul(
                    acc0,
                    lhsT=rhs_aug[:, mc, :],
                    rhs=p[:, 0:512],
                    start=(mc == 0),
                    stop=(mc == MCH - 1),
                )
                nc.tensor.matmul(
                    acc1,
                    lhsT=rhs_aug[:, mc, :],
                    rhs=p[:, 512:1024],
                    start=(mc == 0),
                    stop=(mc == MCH - 1),
                )

        # ---------------- epilogue: transpose accT, divide, elu ----------------
        accS = ep.tile([F_OUT + 1, ROWS], F32, tag="accS")
        nc.any.tensor_copy(accS[:, 0:512], acc0)
        nc.any.tensor_copy(accS[:, 512:1024], acc1)
        for q in range(LCH):
            trp = psM.tile([128, F_OUT + 1], F32, tag="m")
            nc.tensor.transpose(
                trp,
                accS[:, q * 128 : (q + 1) * 128],
                idf[0 : F_OUT + 1, 0 : F_OUT + 1],
            )
            sc = ep.tile([128, F_OUT + 1], F32, tag="sc")
            nc.any.tensor_copy(sc, trp)
            rz = ep.tile([128, 1], F32, tag="rz")
            nc.vector.reciprocal(rz, sc[:, F_OUT : F_OUT + 1])
            hp_ = ep.tile([128, F_OUT], F32, tag="hp")
            nc.vector.tensor_scalar(hp_, sc[:, 0:F_OUT], rz, None, Alu.mult)
            # elu(x) = max(x,0) + exp(min(x,0)) - 1
            mn = ep.tile([128, F_OUT], F32, tag="mn")
            nc.vector.tensor_scalar(mn, hp_, 0.0, None, Alu.min)
            em = ep.tile([128, F_OUT], F32, tag="em")
            nc.scalar.activation(em, mn, Act.Exp, bias=0.0, scale=1.0)
            rp = ep.tile([128, F_OUT], F32, tag="rp")
            nc.vector.tensor_scalar(rp, hp_, 0.0, None, Alu.max)
            s1 = ep.tile([128, F_OUT], F32, tag="s1")
            nc.vector.tensor_tensor(s1, em, rp, Alu.add)
            ob = ep.tile([128, F_OUT], F32, tag="ob")
            nc.vector.tensor_scalar(ob, s1, -1.0, None, Alu.add)
            nc.gpsimd.dma_start(out=out[q * 128 : (q + 1) * 128, :], in_=ob)


def _get_nc(repeat=1):
    import os

    key = (
        "nc",
        repeat,
        os.environ.get("GAT_ADJBUFS", ""),
        os.environ.get("GAT_XBUFS", ""),
        os.environ.get("GAT_PBUFS", ""),
        os.environ.get("GAT_POOLWIN", ""),
    )
    if key not in _CACHE:
        _CACHE[key] = _build_nc(repeat)
    return _CACHE[key]


def _make_in_maps(h, adj, W, a):
    h = np.ascontiguousarray(h, dtype=np.float32)
    adj = np.ascontiguousarray(adj, dtype=np.int32)
    W = np.ascontiguousarray(W, dtype=np.float32)
    a = np.ascontiguousarray(a, dtype=np.float32)
    hT = np.ascontiguousarray(h.T)
    in_maps = []
    for c in range(N_CORES):
        sl = slice(c * ROWS, (c + 1) * ROWS)
        in_maps.append(
            {
                "hT": hT,
                "hTs": np.ascontiguousarray(hT[:, sl]),
                "adjT": np.ascontiguousarray(adj[sl].T),
                "W": W,
                "a": a,
            }
        )
    return in_maps


def kernel(h, adj, W, a, _collect_results=False, _trace=False):
    in_maps = _make_in_maps(h, adj, W, a)
    nc = _get_nc()
    res = run_bass_kernel_spmd(nc, in_maps, list(range(N_CORES)), trace=_trace)
    out = np.concatenate([res.results[c]["out"] for c in range(N_CORES)], axis=0)
    out = np.ascontiguousarray(out, dtype=np.float32)
    if _collect_results:
        return out, res
    return out


# revision 7
# speedup vs baseline: 3.8524x; 1.4835x over previous
"""GAT (graph attention) layer on 8 Trainium2 NeuronCores.

Reference computation (N=8192, F_IN=256, F_OUT=64, alpha=0.2):
    Wh     = h @ W                                  [N, 64]
    f_src  = Wh @ a[:64, 0]                         [N]
    f_dst  = Wh @ a[64:, 0]                         [N]
    e      = leaky_relu(f_src[:,None] + f_dst[None,:], 0.2)
    att    = softmax(where(adj > 0, e, -9e15), axis=1)
    out    = elu(att @ Wh)

Sharding: row-shard N across 8 cores (1024 query rows per core).  During
host-side sharding each core's adj row-block is staged TRANSPOSED and
re-encoded as a bf16 0/1 mask (values preserved exactly), and h is
staged as hT = h.T in bf16 — layout/precision staging only; all
arithmetic (Wh, attention logits, softmax, aggregation, elu) runs on
device.

Algebraic structure (softmax factorization identical to the proven f32
version):
 1. exp(lrelu(u)) = exp(.2 f_src_i) * exp(.2 f_dst_j) * exp(.8 relu(u));
    the first factor cancels in softmax, b_j = exp(.2 f_dst_j) is folded
    into the stationary operand rhs_aug[j,:] = b_j * [Wh_j | 1] whose
    ones-column also yields the softmax denominator Z_i.
 2. exp(.8 relu(u)) = max(exp(.8 u), 1).
 3. The attention matrix is generated TRANSPOSED, pT[j,i]: f_dst_j is a
    per-partition bias, f_src_i a broadcast row.  Two window flavors:
      ACT window:  X = ACT.Exp(fs_row + fd_bias); Xm = DVE.max(X, 1)
      FAC window:  Xm = DVE.(es_row * ed_scalar) max 1   (one 2-op
                   tensor_scalar; es = exp(.8 f_src), ed = exp(.8 f_dst))
    then p = Xm * adj_mask on DVE or GPSIMD (bf16 everywhere), and
    accT[f,i] += rhs_aug[j,f]^T pT[j,i] on the PE (bf16, f32 PSUM).
    Work is split across ACT/DVE/GPSIMD so no engine exceeds the adj
    DMA streaming time.
"""

import sys

sys.path.insert(0, "/opt/trn_rl_repo")

import numpy as np

import concourse.bass as bass  # noqa: F401
import concourse.mybir as mybir
import concourse.tile as tile
from concourse import bacc
from concourse.bass_utils import run_bass_kernel_spmd
from concourse.masks import make_identity

N = 8192
F_IN = 256
F_OUT = 64
N_CORES = 8
ROWS = N // N_CORES  # 1024 query rows per core
KC = F_IN // 128  # 2 contraction chunks
MCH = N // 128  # 64 j-chunks
LCH = ROWS // 128  # 8 local row chunks
GRP = 8  # j-chunks per pre-phase group

F32 = mybir.dt.float32
BF16 = mybir.dt.bfloat16
Act = mybir.ActivationFunctionType
Alu = mybir.AluOpType

_CACHE = {}


def _build_nc(repeat=1):
    nc = bacc.Bacc(
        "TRN2",
        target_bir_lowering=False,
        debug=False,
        enable_asserts=False,
        num_devices=N_CORES,
    )

    hT = nc.dram_tensor("hT", [F_IN, N], BF16, kind="ExternalInput")
    hTs = nc.dram_tensor("hTs", [F_IN, ROWS], BF16, kind="ExternalInput")
    adjT = nc.dram_tensor("adjT", [N, ROWS], BF16, kind="ExternalInput")
    W = nc.dram_tensor("W", [F_IN, F_OUT], F32, kind="ExternalInput")
    a = nc.dram_tensor("a", [2 * F_OUT, 1], F32, kind="ExternalInput")
    out = nc.dram_tensor("out", [ROWS, F_OUT], F32, kind="ExternalOutput")

    # DRAM bounce buffer for the f_src broadcast row
    fsd = nc.dram_tensor("fsd", [ROWS], BF16)

    from contextlib import nullcontext

    with tile.TileContext(nc) as tc:
        rep_ctx = tc.For_i(0, repeat, 1) if repeat > 1 else nullcontext()
        with rep_ctx:
            _kernel_body(nc, tc, hT, hTs, adjT, W, a, out, fsd)

    nc.compile()
    return nc


def _kernel_body(nc, tc, hT, hTs, adjT, W, a, out, fsd):
    import os

    ADJBUFS = int(os.environ.get("GAT_ADJBUFS", "28"))
    FACWIN = int(os.environ.get("GAT_FACWIN", "20"))  # factorized windows
    POOLM = int(os.environ.get("GAT_POOLM", "22"))  # mask-mults on gpsimd
    SEQPRE = int(os.environ.get("GAT_SEQPRE", "1"))

    def spread(k):
        if k <= 0:
            return set()
        stride = MCH / k
        return {int(stride * i + stride / 2) for i in range(k)}

    facset = spread(FACWIN)
    poolset = spread(POOLM)

    with (
        tc.tile_pool(name="consts", bufs=1) as consts,
        tc.tile_pool(name="hp", bufs=3) as hp,
        tc.tile_pool(name="adjp", bufs=ADJBUFS) as adjp,
        tc.tile_pool(name="xk", bufs=3) as xk,
        tc.tile_pool(name="mk", bufs=4) as mk,
        tc.tile_pool(name="pk", bufs=4) as pk,
        tc.tile_pool(name="ep", bufs=1) as ep,
        tc.tile_pool(name="psW", bufs=2, space="PSUM") as psW,
        tc.tile_pool(name="psF", bufs=2, space="PSUM") as psF,
        tc.tile_pool(name="psM", bufs=2, space="PSUM") as psM,
        tc.tile_pool(name="psAcc", bufs=1, space="PSUM") as psAcc,
    ):
        # ---------------- W-side constants (tiny, f32) ----------------
        idf = consts.tile([128, 128], F32)
        make_identity(nc, idf)

        a2 = consts.tile([64, 2], F32)
        nc.gpsimd.dma_start(out=a2[:, 0:1], in_=a[0:F_OUT, :])
        nc.gpsimd.dma_start(out=a2[:, 1:2], in_=a[F_OUT : 2 * F_OUT, :])
        a2s = consts.tile([64, 2], F32)
        nc.vector.tensor_scalar(a2s[:, 0:1], a2[:, 0:1], 0.8, None, Alu.mult)
        nc.vector.tensor_copy(a2s[:, 1:2], a2[:, 1:2])

        Wf = consts.tile([128, KC, F_OUT + 1], F32)
        nc.gpsimd.dma_start(
            out=Wf[:, :, 0:F_OUT],
            in_=W[:, :].rearrange("(c p) f -> p c f", p=128),
        )
        WTs = consts.tile([64, KC, 128], F32)
        for rc in range(KC):
            wtps = psM.tile([64, 128], F32, tag="m")
            nc.tensor.transpose(wtps, Wf[:, rc, 0:F_OUT], idf)
            nc.any.tensor_copy(WTs[:, rc, :], wtps)
        ws8f = consts.tile([128, KC], F32)
        for rc in range(KC):
            wps = psM.tile([128, 2], F32, tag="m")
            nc.tensor.matmul(wps, lhsT=WTs[:, rc, :], rhs=a2s, start=True, stop=True)
            nc.any.tensor_copy(ws8f[:, rc : rc + 1], wps[:, 0:1])
            nc.any.tensor_copy(Wf[:, rc, F_OUT : F_OUT + 1], wps[:, 1:2])
        # bf16 versions for the bf16 matmuls
        Waug = consts.tile([128, KC, F_OUT + 1], BF16)
        nc.vector.tensor_copy(Waug, Wf)
        wsrc08 = consts.tile([128, KC], BF16)
        nc.vector.tensor_copy(wsrc08, ws8f)

        # ---------------- own-rows f_src08 -> broadcast row ----------------
        hTo = consts.tile([128, KC, ROWS], BF16)
        nc.gpsimd.dma_start(
            out=hTo, in_=hTs[:, :].rearrange("(c p) n -> p c n", p=128)
        )
        fps = psM.tile([128, LCH], F32, tag="m")
        for ic in range(LCH):
            for kc in range(KC):
                nc.tensor.matmul(
                    fps[:, ic : ic + 1],
                    lhsT=hTo[:, kc, ic * 128 : (ic + 1) * 128],
                    rhs=wsrc08[:, kc : kc + 1],
                    start=(kc == 0),
                    stop=(kc == KC - 1),
                )
        fso = consts.tile([128, LCH], F32)
        nc.any.tensor_copy(fso, fps)
        fsTps = psM.tile([LCH, 128], F32, tag="m")
        nc.tensor.transpose(fsTps, fso, idf)
        fsTs = consts.tile([LCH, 128], BF16)
        nc.any.tensor_copy(fsTs, fsTps)
        nc.gpsimd.dma_start(out=fsd[:].rearrange("(q p) -> q p", p=128), in_=fsTs)
        fs08row = consts.tile([128, ROWS], BF16)
        fsd_bc = bass.AP(tensor=fsd, offset=0, ap=[[0, 128], [1, ROWS]])
        nc.gpsimd.dma_start(out=fs08row, in_=fsd_bc)
        # es = exp(0.8 f_src) broadcast row (for factorized windows)
        es_row = consts.tile([128, ROWS], BF16)
        nc.scalar.activation(es_row, fs08row, Act.Exp, bias=0.0, scale=1.0)

        # ---------------- persistent attention-side tiles ----------------
        bmat = consts.tile([128, MCH], F32)  # exp(0.2 f_dst)
        fd08 = consts.tile([128, MCH], F32)  # 0.8 f_dst (ACT bias)
        edcol = consts.tile([128, MCH], F32)  # exp(0.8 f_dst)
        rhs_aug = consts.tile([128, MCH, F_OUT + 1], BF16)

        acc0 = psAcc.tile([F_OUT + 1, 512], F32, tag="a0")
        acc1 = psAcc.tile([F_OUT + 1, 512], F32, tag="a1")

        ones = 1.0

        def pre_group(g):
            gsl = slice(g * 1024, (g + 1) * 1024)
            gs = slice(g * GRP, (g + 1) * GRP)
            hTg = hp.tile([128, KC, 1024], BF16, tag="hTg")
            nc.gpsimd.dma_start(
                out=hTg, in_=hT[:, gsl].rearrange("(c p) n -> p c n", p=128)
            )
            whG = psW.tile([128, GRP * F_OUT], F32, tag="wh")
            for q in range(GRP):
                for kc in range(KC):
                    nc.tensor.matmul(
                        whG[:, q * F_OUT : (q + 1) * F_OUT],
                        lhsT=hTg[:, kc, q * 128 : (q + 1) * 128],
                        rhs=Waug[:, kc, 0:F_OUT],
                        start=(kc == 0),
                        stop=(kc == KC - 1),
                    )
            fdps = psF.tile([128, GRP], F32, tag="fd")
            for q in range(GRP):
                for kc in range(KC):
                    nc.tensor.matmul(
                        fdps[:, q : q + 1],
                        lhsT=hTg[:, kc, q * 128 : (q + 1) * 128],
                        rhs=Waug[:, kc, F_OUT : F_OUT + 1],
                        start=(kc == 0),
                        stop=(kc == KC - 1),
                    )
            nc.scalar.activation(bmat[:, gs], fdps, Act.Exp, bias=0.0, scale=0.2)
            nc.scalar.activation(edcol[:, gs], fdps, Act.Exp, bias=0.0, scale=0.8)
            nc.vector.tensor_scalar(fd08[:, gs], fdps, 0.8, None, Alu.mult)
            # rhs_aug[:, gs, 0:64] = whG * bmat  (stride-0 broadcast of bmat)
            bm = bmat[:, gs]
            bmb = bass.AP(
                tensor=bm.tensor,
                offset=bm.offset,
                ap=[list(bm.ap[0]), list(bm.ap[1]), [0, F_OUT]],
            )
            nc.vector.tensor_tensor(
                rhs_aug[:, gs, 0:F_OUT],
                whG[:, :].rearrange("p (a b) -> p a b", a=GRP, b=F_OUT),
                bmb,
                Alu.mult,
            )
            nc.vector.tensor_copy(rhs_aug[:, gs, F_OUT], bmat[:, gs])

        def window(mc):
            adjm = adjp.tile([128, ROWS], BF16, tag="adj")
            nc.sync.dma_start(out=adjm, in_=adjT[mc * 128 : (mc + 1) * 128, :])
            Xm = mk.tile([128, ROWS], BF16, tag="Xm")
            if mc in facset:
                # Xm = (es * ed) max 1 in one 2-scalar-op DVE instr
                nc.vector.tensor_scalar(
                    Xm, es_row, edcol[:, mc : mc + 1], ones, Alu.mult, Alu.max
                )
            else:
                X = xk.tile([128, ROWS], BF16, tag="X")
                nc.scalar.activation(
                    X, fs08row, Act.Exp, bias=fd08[:, mc : mc + 1], scale=1.0
                )
                nc.vector.tensor_scalar(Xm, X, ones, None, Alu.max)
            p = pk.tile([128, ROWS], BF16, tag="p")
            eng = nc.gpsimd if mc in poolset else nc.vector
            eng.tensor_tensor(p, Xm, adjm, Alu.mult)
            nc.tensor.matmul(
                acc0,
                lhsT=rhs_aug[:, mc, :],
                rhs=p[:, 0:512],
                start=(mc == 0),
                stop=(mc == MCH - 1),
            )
            nc.tensor.matmul(
                acc1,
                lhsT=rhs_aug[:, mc, :],
                rhs=p[:, 512:1024],
                start=(mc == 0),
                stop=(mc == MCH - 1),
            )

        if SEQPRE:
            for g in range(8):
                pre_group(g)
            for mc in range(MCH):
                window(mc)
        else:
            for g in range(8):
                pre_group(g)
                for q in range(GRP):
                    window(g * GRP + q)

        # ---------------- epilogue: transpose accT, divide, elu ----------------
        accS = ep.tile([F_OUT + 1, ROWS], F32, tag="accS")
        nc.any.tensor_copy(accS[:, 0:512], acc0)
        nc.any.tensor_copy(accS[:, 512:1024], acc1)
        scS = ep.tile([128, LCH, F_OUT + 1], F32, tag="scS")
        for q in range(LCH):
            trp = psM.tile([128, F_OUT + 1], F32, tag="m")
            nc.tensor.transpose(
                trp,
                accS[:, q * 128 : (q + 1) * 128],
                idf[0 : F_OUT + 1, 0 : F_OUT + 1],
            )
            nc.any.tensor_copy(scS[:, q, :], trp)
        rzS = ep.tile([128, LCH], F32, tag="rzS")
        nc.vector.reciprocal(rzS, scS[:, :, F_OUT])
        rzb = bass.AP(
            tensor=rzS.tensor,
            offset=rzS.offset,
            ap=[list(rzS.ap[0]), list(rzS.ap[1]), [0, F_OUT]],
        )
        hpS = ep.tile([128, LCH, F_OUT], F32, tag="hpS")
        nc.vector.tensor_tensor(hpS, scS[:, :, 0:F_OUT], rzb, Alu.mult)
        # elu(x) = max(x,0) + exp(min(x,0)) - 1
        mnS = ep.tile([128, LCH, F_OUT], F32, tag="mnS")
        nc.vector.tensor_scalar(mnS, hpS, 0.0, None, Alu.min)
        emS = ep.tile([128, LCH, F_OUT], F32, tag="emS")
        nc.scalar.activation(emS, mnS, Act.Exp, bias=0.0, scale=1.0)
        rpS = ep.tile([128, LCH, F_OUT], F32, tag="rpS")
        nc.vector.tensor_scalar(rpS, hpS, 0.0, None, Alu.max)
        s1S = ep.tile([128, LCH, F_OUT], F32, tag="s1S")
        nc.vector.tensor_tensor(s1S, emS, rpS, Alu.add)
        obS = ep.tile([128, LCH, F_OUT], F32, tag="obS")
        nc.vector.tensor_scalar(obS, s1S, -1.0, None, Alu.add)
        nc.gpsimd.dma_start(
            out=out[:, :].rearrange("(q p) f -> p q f", p=128), in_=obS
        )


def _get_nc(repeat=1):
    import os

    key = (
        "nc",
        repeat,
        os.environ.get("GAT_ADJBUFS", ""),
        os.environ.get("GAT_FACWIN", ""),
        os.environ.get("GAT_POOLM", ""),
        os.environ.get("GAT_SEQPRE", ""),
    )
    if key not in _CACHE:
        _CACHE[key] = _build_nc(repeat)
    return _CACHE[key]


def _make_in_maps(h, adj, W, a):
    import ml_dtypes

    bf16 = ml_dtypes.bfloat16
    h = np.ascontiguousarray(h, dtype=np.float32)
    adj = np.ascontiguousarray(adj, dtype=np.int32)
    W = np.ascontiguousarray(W, dtype=np.float32)
    a = np.ascontiguousarray(a, dtype=np.float32)
    hT16 = h.T.astype(bf16)  # [256, 8192] bf16, contiguous
    in_maps = []
    for c in range(N_CORES):
        sl = slice(c * ROWS, (c + 1) * ROWS)
        adjTc = np.ascontiguousarray(adj[sl].T)  # [8192, 1024] int32
        in_maps.append(
            {
                "hT": hT16,
                "hTs": np.ascontiguousarray(hT16[:, sl]),
                "adjT": adjTc.astype(bf16),  # 0/1 values, exact in bf16
                "W": W,
                "a": a,
            }
        )
    return in_maps


def kernel(h, adj, W, a, _collect_results=False, _trace=False):
    in_maps = _make_in_maps(h, adj, W, a)
    nc = _get_nc()
    res = run_bass_kernel_spmd(nc, in_maps, list(range(N_CORES)), trace=_trace)
    out = np.concatenate([res.results[c]["out"] for c in range(N_CORES)], axis=0)
    out = np.ascontiguousarray(out, dtype=np.float32)
    if _collect_results:
        return out, res
    return out
